# revision 22
# baseline (speedup 1.0000x reference)
"""DeeperGCN on 8 TRN2 NeuronCores via Bass/Tile.

Sharding: nodes in contiguous ranges of 12500/core (padded to 12544), edges
assigned to the dst-owner core. Per layer: BatchNorm stats via tiny
AllReduce, h2 AllGathered in fp16 with the 8 bond-embedding combo rows
riding along, per-edge h2+emb rows fetched in one merged indirect DMA into
degree-grouped slot tiles, scatter-softmax with fp32 exp/tree-reductions
(u reaches ~29, so fp16/bf16 exp would overflow), MLP in fp16 weights /
activations with fp32 PSUM accumulation. Atom encoding, final BN, and
graph pooling run on the host.
"""
import sys
sys.path.insert(0, '/opt/trn_rl_repo')

import numpy as np
import ml_dtypes

N, E, G, D, H = 100000, 400000, 4096, 128, 256
NL = 20
NCORES = 8
SH = 12500          # real nodes per core
SP = 12544          # padded nodes per core (98 blocks of 128)
NB = SP // 128      # 98 node blocks
TPC = 12560         # table rows per core: SP nodes + 8 emb rows + dummy + pad
TROWS = TPC * NCORES
EMB_ROW = SP        # rows SP..SP+7 of core0's section hold the 8 bond combos
DUMMY_ROW = SP + 8  # global row index of core 0's dummy row
DUMMY_VAL = -100.0
EPS = 1e-7
BN_EPS = 1e-5

F16 = np.float16

_CACHE = {}
LAST_HW_EXEC_NS = None


# ---------------------------------------------------------------- host prep

def _preprocess(x, edge_index, edge_attr, atom_emb):
    """Per-core gather offsets (h2 + emb merged), empty-slot counts, and
    permuted h0 shards."""
    src, dst = edge_index[0], edge_index[1]
    m_e = (edge_attr[:, 0] + 2 * edge_attr[:, 1] + 4 * edge_attr[:, 2]).astype(np.int64)
    h0 = atom_emb[0][x[:, 0]].astype(np.float32).copy()
    for i in range(1, 9):
        h0 += atom_emb[i][x[:, i]]

    owner = dst // SH
    perms = []
    inv_pos = np.empty(N, np.int64)   # node-global-id -> row in gathered table
    degs_c = []
    for c in range(NCORES):
        deg = np.bincount(dst[owner == c] - c * SH, minlength=SH)
        order = np.argsort(-deg, kind='stable')
        perms.append(order)
        inv = np.empty(SH, np.int64)
        inv[order] = np.arange(SH)
        inv_pos[c * SH:(c + 1) * SH] = c * TPC + inv
        degs_c.append(deg[order])

    ksched = np.zeros(NB, np.int64)
    for c in range(NCORES):
        dp = np.concatenate([degs_c[c], np.zeros(SP - SH, np.int64)])
        ksched = np.maximum(ksched, dp.reshape(NB, 128).max(1))
    ksched = ksched.astype(np.int64)
    nchunk = int(ksched.sum())

    offs2s, ecnts, h0Ts = [], [], []
    for c in range(NCORES):
        mask = owner == c
        eidx = np.nonzero(mask)[0]
        d_loc = dst[eidx] - c * SH
        pos = np.empty(SH, np.int64)
        pos[perms[c]] = np.arange(SH)
        lane_of_edge = pos[d_loc]
        o = np.argsort(lane_of_edge, kind='stable')
        eidx, lane_of_edge = eidx[o], lane_of_edge[o]
        counts = np.bincount(lane_of_edge, minlength=SP)
        starts = np.concatenate([[0], np.cumsum(counts)[:-1]])

        off2 = np.full((128, 2 * nchunk), DUMMY_ROW, np.int32)
        ecnt = np.zeros((128, NB), np.float32)
        bp = 0
        for p in range(NB):
            kp = int(ksched[p])
            lanes = np.arange(128)
            glob = p * 128 + lanes
            ecnt[:, p] = np.maximum(kp - counts[glob], 0)
            for k in range(kp):
                has = counts[glob] > k
                hg = glob[has]
                ee = eidx[starts[hg] + k]
                off2[has, bp + k] = inv_pos[src[ee]]
                off2[has, bp + kp + k] = EMB_ROW + m_e[ee]
            bp += 2 * kp
        assert bp == 2 * nchunk
        offs2s.append(off2)
        ecnts.append(ecnt)
        hp = np.zeros((SP, D), np.float32)
        hp[:SH] = h0[c * SH:(c + 1) * SH][perms[c]]
        h0Ts.append(np.ascontiguousarray(hp.T))

    return dict(ksched=ksched, nchunk=nchunk, offs2=offs2s, ecnt=ecnts,
                h0T=h0Ts, perms=perms)


def _pack_params(bond_emb, W1, W2, W3, b1, g1, be1, b2, g2, be2, b3,
                 norm_g, norm_b, nl):
    # weights fp16, [128, nl*1024]: per layer W1|W2k0|W2k1|W3k0|W3k1
    wcols = []
    for l in range(nl):
        wcols += [W1[l], W2[l][:128], W2[l][128:], W3[l][:128], W3[l][128:]]
    wblob = np.concatenate(wcols, axis=1).astype(F16)
    # emb8: 8 bond combos per layer, [nl*8, 128] fp16
    rows = []
    for l in range(nl):
        for m in range(8):
            rows.append(bond_emb[l, 0, m & 1] + bond_emb[l, 1, (m >> 1) & 1]
                        + bond_emb[l, 2, (m >> 2) & 1])
    emb8 = np.stack(rows).astype(F16)
    # bn/bias params fp32 [128, nl*16]
    pcols = []
    for l in range(nl):
        for v in (b1[l], g1[l], be1[l]):
            pcols += [v[:128], v[128:]]
        for v in (b2[l], g2[l], be2[l]):
            pcols += [v[:128], v[128:]]
        pcols += [b3[l], norm_g[l], norm_b[l], np.zeros(128, np.float32)]
    pblob = np.stack(pcols, axis=1).astype(np.float32)
    return wblob, emb8, pblob


# ---------------------------------------------------------------- device

def _build(nl, nchunk, ksched, t_vals):
    import concourse.bass as bass
    import concourse.mybir as mybir
    import concourse.tile as tile
    import concourse.bacc as bacc
    from concourse.masks import make_identity

    AF = mybir.ActivationFunctionType
    OP = mybir.AluOpType
    f32, f16, i32 = mybir.dt.float32, mybir.dt.float16, mybir.dt.int32

    kmax = int(max(ksched))
    nc = bacc.Bacc("TRN2", target_bir_lowering=False, debug=False)

    h0T = nc.declare_dram_parameter("h0T", [D, SP], f16, isOutput=False)
    offs = nc.declare_dram_parameter("offs", [128, 2 * nchunk], i32, isOutput=False)
    ecnt_d = nc.declare_dram_parameter("ecnt", [128, NB], f32, isOutput=False)
    wblob_d = nc.declare_dram_parameter("wblob", [128, nl * 1024], f16, isOutput=False)
    emb8_d = nc.declare_dram_parameter("emb8", [nl * 8, D], f16, isOutput=False)
    pblob_d = nc.declare_dram_parameter("pblob", [128, nl * 16], f32, isOutput=False)
    hT_out = nc.declare_dram_parameter("hT_out", [D, SP], f16, isOutput=True)
    DEBUG = bool(int(__import__("os").environ.get("K2_DEBUG", "0")))
    if DEBUG:
        dbg1 = nc.declare_dram_parameter("dbg1", [48, D], f16, isOutput=True)
        dbg2 = nc.declare_dram_parameter("dbg2", [128, 512], f16, isOutput=True)
        dbg3 = nc.declare_dram_parameter("dbg3", [128, 128], f32, isOutput=True)

    with tile.TileContext(nc) as tc:
        with tc.tile_pool(name="static", bufs=1) as stp, \
             tc.tile_pool(name="state", bufs=1) as sta, \
             tc.tile_pool(name="work", bufs=3) as wk, \
             tc.tile_pool(name="slotp", bufs=2) as slp, \
             tc.tile_pool(name="psA", bufs=2, space="PSUM") as psA, \
             tc.tile_pool(name="psB", bufs=2, space="PSUM") as psB, \
             tc.tile_pool(name="dram", bufs=2, space="DRAM") as dram:

            # ---- static loads
            off_t = stp.tile([128, 2 * nchunk], i32)
            nc.sync.dma_start(out=off_t[:], in_=offs[:])
            ecnt_t = stp.tile([128, NB], f32)
            nc.sync.dma_start(out=ecnt_t[:], in_=ecnt_d[:])
            pb = stp.tile([128, nl * 16], f32)
            nc.sync.dma_start(out=pb[:], in_=pblob_d[:])
            ident = stp.tile([128, 128], f32)
            make_identity(nc, ident[:])
            bneps_t = stp.tile([128, 1], f32)
            nc.vector.memset(bneps_t[:], BN_EPS)
            dummy_t = stp.tile([8, 128], f16)
            nc.vector.memset(dummy_t[:], DUMMY_VAL)

            # ---- state
            h_c = sta.tile([D, SP], f32)
            nc.gpsimd.dma_start(out=h_c[:], in_=h0T[:])
            hh = sta.tile([D, SP], f32)            # h2 then hh, per layer
            hh16 = sta.tile([D, SP], f16)          # fp16 copy for matmuls
            ecnt_l = sta.tile([128, NB], f32)
            stb_n = sta.tile([128, 25 * 6], f32)   # next layer's h stats (pass C)

            # per-layer DRAM bounce tiles
            h2b = dram.tile([TPC, D], f16, tag="h2b")
            stin = dram.tile([128, 4], f32, tag="stin")

            MT = [(i * 512, min(512, SP - i * 512)) for i in range((SP + 511) // 512)]
            ST = [(i * 512, min(512, SH - i * 512)) for i in range((SH + 511) // 512)]

            def allreduce_stats(pack):
                nc.sync.dma_start(out=stin[:], in_=pack[:])
                stout = dram.tile([128, 4], f32, tag="stout",
                                  addr_space="Shared")
                nc.gpsimd.collective_compute(
                    "AllReduce", OP.add, replica_groups=[list(range(NCORES))],
                    ins=[stin.opt()], outs=[stout.opt()])
                res = wk.tile([128, 4], f32, tag="arres")
                nc.sync.dma_start(out=res[:], in_=stout[:])
                return res

            def col(lay, j):
                return pb[:, lay * 16 + j:lay * 16 + j + 1]

            for l in range(nl):
                t_l = float(t_vals[l])
                c0_l = float(np.exp(t_l * EPS))
                wbuf = wk.tile([128, 1024], f16, tag="wbuf", bufs=2)
                nc.sync.dma_start(out=wbuf[:], in_=wblob_d[:, l * 1024:(l + 1) * 1024])
                # emb rows + dummy row into the bounce table
                nc.sync.dma_start(out=h2b[SP:SP + 8, :], in_=emb8_d[l * 8:(l + 1) * 8, :])
                nc.sync.dma_start(out=h2b[SP + 8:TPC, :], in_=dummy_t[:])

                # ---------- pre-norm -> hh ( = h2 )
                if l == 0:
                    for s, w in MT:
                        nc.vector.tensor_copy(hh[:, s:s + w], h_c[:, s:s + w])
                else:
                    agg = wk.tile([128, 2], f32, tag="agg")
                    nc.vector.bn_aggr(agg[:], stb_n[:])
                    pack = wk.tile([128, 4], f32, tag="pack")
                    nc.vector.memset(pack[:, 2:4], 0.0)
                    nc.vector.tensor_scalar_mul(pack[:, 0:1], agg[:, 0:1], float(SH))
                    sq = wk.tile([128, 1], f32, tag="sq")
                    nc.scalar.square(sq[:], agg[:, 0:1])
                    nc.vector.tensor_add(pack[:, 1:2], agg[:, 1:2], sq[:])
                    nc.vector.tensor_scalar_mul(pack[:, 1:2], pack[:, 1:2], float(SH))
                    res = allreduce_stats(pack)
                    mu = wk.tile([128, 1], f32, tag="mu")
                    nc.vector.tensor_scalar_mul(mu[:], res[:, 0:1], 1.0 / N)
                    var = wk.tile([128, 1], f32, tag="var")
                    nc.vector.tensor_scalar_mul(var[:], res[:, 1:2], 1.0 / N)
                    nc.scalar.square(sq[:], mu[:])
                    nc.vector.tensor_tensor(out=var[:], in0=var[:], in1=sq[:], op=OP.subtract)
                    sd = wk.tile([128, 1], f32, tag="sd")
                    nc.scalar.activation(sd[:], var[:], AF.Sqrt, bias=bneps_t[:])
                    rinv = wk.tile([128, 1], f32, tag="rinv")
                    nc.vector.reciprocal(rinv[:], sd[:])
                    scale = wk.tile([128, 1], f32, tag="scale")
                    nc.vector.tensor_mul(scale[:], rinv[:], col(l - 1, 13))
                    nscale = wk.tile([128, 1], f32, tag="nscale")
                    nc.vector.tensor_scalar_mul(nscale[:], scale[:], -1.0)
                    bias = wk.tile([128, 1], f32, tag="bias")
                    nc.vector.scalar_tensor_tensor(
                        out=bias[:], in0=mu[:], scalar=nscale[:], in1=col(l - 1, 14),
                        op0=OP.mult, op1=OP.add)
                    for s, w in MT:
                        nc.scalar.activation(hh[:, s:s + w], h_c[:, s:s + w],
                                             AF.Relu, bias=bias[:], scale=scale[:])

                # ---------- h2 bounce (fp16 row-major) and AllGather
                stg_n = 7
                for q0 in range(0, NB, stg_n):
                    qn = min(stg_n, NB - q0)
                    stg = wk.tile([128, stg_n * 128], f16, tag="stg")
                    for j in range(qn):
                        bidx = q0 + j
                        ps = psA.tile([128, 128], f32, tag="tr", name="pst")
                        nc.tensor.transpose(
                            out=ps[:], in_=hh[:, bidx * 128:(bidx + 1) * 128],
                            identity=ident[:])
                        nc.scalar.copy(stg[:, j * 128:(j + 1) * 128], ps[:])
                    nc.sync.dma_start(
                        out=h2b[q0 * 128:(q0 + qn) * 128, :]
                            .rearrange("(q n) c -> n q c", q=qn),
                        in_=stg[:, :qn * 128].rearrange("n (q c) -> n q c", q=qn))
                h2full = dram.tile([TROWS, D], f16, tag="h2full",
                                   addr_space="Shared")
                nc.gpsimd.collective_compute(
                    "AllGather", OP.bypass, replica_groups=[list(range(NCORES))],
                    ins=[h2b.opt()], outs=[h2full.opt()])
                if DEBUG and l == 0:
                    nc.sync.dma_start(out=dbg1[0:32, :], in_=h2full[0:32, :])
                    nc.sync.dma_start(out=dbg1[32:48, :], in_=h2full[SP:SP + 16, :])

                # empty-slot denom correction: ecnt * exp(t*eps)
                nc.vector.tensor_scalar_mul(ecnt_l[:], ecnt_t[:], c0_l)
                teps_t = wk.tile([128, 1], f32, tag="teps")
                nc.vector.memset(teps_t[:], t_l * EPS)

                # ---------- messages per block
                ci = 0
                for p in range(NB):
                    kp = int(ksched[p])
                    blk = slice(p * 128, (p + 1) * 128)
                    if kp == 0:
                        nc.vector.tensor_scalar_add(hh[:, blk], hh[:, blk], EPS)
                        nc.scalar.copy(hh16[:, blk], hh[:, blk])
                        continue
                    slots = slp.tile([128, 2 * kmax * 128], f16, tag="slots")
                    for k in range(2 * kp):
                        nc.gpsimd.indirect_dma_start(
                            out=slots[:, k * 128:(k + 1) * 128], out_offset=None,
                            in_=h2full[:],
                            in_offset=bass.IndirectOffsetOnAxis(
                                ap=off_t[:, 2 * ci + k:2 * ci + k + 1], axis=0))
                    if DEBUG and l == 0 and p == 0:
                        dslots = wk.tile([128, 512], f16, tag="dslots")
                        nc.vector.tensor_copy(dslots[:], slots[:, 0:512])
                        nc.sync.dma_start(out=dbg2[:], in_=dslots[:])
                    g = slots[:, :kp * 128]
                    with nc.allow_low_precision("fp16 edge stream"):
                        nc.vector.tensor_tensor(
                            out=g, in0=g, in1=slots[:, kp * 128:2 * kp * 128], op=OP.add)
                        nc.vector.tensor_scalar_max(g, g, 0.0)    # u
                    e1 = slp.tile([128, kmax * 128], f32, tag="e1")
                    nc.scalar.activation(e1[:, :kp * 128], g, AF.Exp,
                                         bias=teps_t[:], scale=t_l)
                    qt = slp.tile([128, kmax * 128], f32, tag="qt")
                    nc.vector.tensor_tensor(out=qt[:, :kp * 128], in0=e1[:, :kp * 128],
                                            in1=g, op=OP.mult)
                    # in-place halving tree reductions over k
                    for buf in (e1, qt):
                        w = kp
                        while w > 1:
                            hw = w // 2
                            if w % 2:
                                nc.vector.tensor_tensor(
                                    out=buf[:, 0:128], in0=buf[:, 0:128],
                                    in1=buf[:, (w - 1) * 128:w * 128], op=OP.add)
                            nc.vector.tensor_tensor(
                                out=buf[:, 0:hw * 128], in0=buf[:, 0:hw * 128],
                                in1=buf[:, hw * 128:2 * hw * 128], op=OP.add)
                            w = hw
                    dacc = wk.tile([128, 128], f32, tag="dacc")
                    nc.vector.tensor_scalar(
                        out=dacc[:], in0=e1[:, 0:128],
                        scalar1=ecnt_l[:, p:p + 1], scalar2=None,
                        op0=OP.subtract)
                    nc.vector.tensor_scalar_max(dacc[:], dacc[:], 1e-7)
                    rec = wk.tile([128, 128], f32, tag="rec")
                    nc.vector.reciprocal(rec[:], dacc[:])
                    m = wk.tile([128, 128], f32, tag="m")
                    nc.vector.tensor_mul(m[:], qt[:, 0:128], rec[:])
                    if DEBUG and l == 0 and p == 0:
                        nc.sync.dma_start(out=dbg3[:], in_=m[:])
                    ps = psA.tile([128, 128], f32, tag="tr")
                    nc.tensor.transpose(out=ps[:], in_=m[:], identity=ident[:])
                    nc.vector.scalar_tensor_tensor(
                        out=hh[:, blk], in0=ps[:],
                        scalar=EPS, in1=hh[:, blk],
                        op0=OP.add, op1=OP.add)
                    nc.scalar.copy(hh16[:, blk], hh[:, blk])
                    ci += kp

                # ---------- MLP (fp16 weights/acts, recompute scheme)
                W1l = wbuf[:, 0:256]
                W2l = [wbuf[:, 256:512], wbuf[:, 512:768]]
                W3l = [wbuf[:, 768:896], wbuf[:, 896:1024]]

                # pass A: stats of z1 = W1^T @ hh
                stA = wk.tile([128, 2 * len(ST) * 6], f32, tag="stA")
                for i, (s, w) in enumerate(ST):
                    for hf in range(2):
                        ps = psB.tile([128, 512], f32, tag="z1")
                        nc.tensor.matmul(ps[:, :w], lhsT=W1l[:, hf * 128:(hf + 1) * 128],
                                         rhs=hh16[:, s:s + w], start=True, stop=True)
                        nc.vector.bn_stats(
                            stA[:, (i * 2 + hf) * 6:(i * 2 + hf + 1) * 6], ps[:, :w])
                agg1 = [wk.tile([128, 2], f32, tag=f"agg1{hf}", name=f"agg1_{hf}") for hf in range(2)]
                for hf in range(2):
                    nc.vector.bn_aggr(
                        agg1[hf][:],
                        stA[:].rearrange("p (i h s) -> p h i s", h=2, s=6)[:, hf])
                pack = wk.tile([128, 4], f32, tag="pack")
                sq = wk.tile([128, 1], f32, tag="sq")
                for hf in range(2):
                    nc.vector.tensor_scalar_mul(pack[:, hf * 2:hf * 2 + 1],
                                                agg1[hf][:, 0:1], float(SH))
                    nc.scalar.square(sq[:], agg1[hf][:, 0:1])
                    nc.vector.tensor_add(pack[:, hf * 2 + 1:hf * 2 + 2],
                                         agg1[hf][:, 1:2], sq[:])
                    nc.vector.tensor_scalar_mul(pack[:, hf * 2 + 1:hf * 2 + 2],
                                                pack[:, hf * 2 + 1:hf * 2 + 2],
                                                float(SH))
                res = allreduce_stats(pack)
                sc1, bi1 = [], []
                for hf in range(2):
                    mu = wk.tile([128, 1], f32, tag=f"mu1{hf}")
                    nc.vector.tensor_scalar_mul(mu[:], res[:, hf * 2:hf * 2 + 1], 1.0 / N)
                    var = wk.tile([128, 1], f32, tag=f"var1{hf}")
                    nc.vector.tensor_scalar_mul(var[:], res[:, hf * 2 + 1:hf * 2 + 2], 1.0 / N)
                    nc.scalar.square(sq[:], mu[:])
                    nc.vector.tensor_tensor(out=var[:], in0=var[:], in1=sq[:], op=OP.subtract)
                    sd = wk.tile([128, 1], f32, tag=f"sd1{hf}")
                    nc.scalar.activation(sd[:], var[:], AF.Sqrt, bias=bneps_t[:])
                    rinv = wk.tile([128, 1], f32, tag=f"ri1{hf}")
                    nc.vector.reciprocal(rinv[:], sd[:])
                    s_ = wk.tile([128, 1], f32, tag=f"s1{hf}")
                    nc.vector.tensor_mul(s_[:], rinv[:], col(l, 2 + hf))
                    mb = wk.tile([128, 1], f32, tag=f"mb1{hf}")
                    nc.vector.tensor_add(mb[:], mu[:], col(l, 0 + hf))
                    ns = wk.tile([128, 1], f32, tag=f"ns1{hf}")
                    nc.vector.tensor_scalar_mul(ns[:], s_[:], -1.0)
                    b_ = wk.tile([128, 1], f32, tag=f"b1{hf}")
                    nc.vector.scalar_tensor_tensor(
                        out=b_[:], in0=mb[:], scalar=ns[:], in1=col(l, 4 + hf),
                        op0=OP.mult, op1=OP.add)
                    sc1.append(s_)
                    bi1.append(b_)

                # pass B: stats of z2
                stB = wk.tile([128, 2 * len(ST) * 6], f32, tag="stB")
                for i, (s, w) in enumerate(ST):
                    a1t = [None, None]
                    for hf in range(2):
                        ps = psB.tile([128, 512], f32, tag="z1")
                        nc.tensor.matmul(ps[:, :w], lhsT=W1l[:, hf * 128:(hf + 1) * 128],
                                         rhs=hh16[:, s:s + w], start=True, stop=True)
                        at = wk.tile([128, 512], f16, tag=f"a1_{hf}", bufs=2)
                        nc.scalar.activation(at[:, :w], ps[:, :w], AF.Relu,
                                             bias=bi1[hf][:], scale=sc1[hf][:])
                        a1t[hf] = at
                    for ho in range(2):
                        ps = psB.tile([128, 512], f32, tag="z2")
                        for kh in range(2):
                            nc.tensor.matmul(ps[:, :w],
                                             lhsT=W2l[kh][:, ho * 128:(ho + 1) * 128],
                                             rhs=a1t[kh][:, :w],
                                             start=(kh == 0), stop=(kh == 1))
                        nc.vector.bn_stats(
                            stB[:, (i * 2 + ho) * 6:(i * 2 + ho + 1) * 6], ps[:, :w])
                for hf in range(2):
                    nc.vector.bn_aggr(
                        agg1[hf][:],
                        stB[:].rearrange("p (i h s) -> p h i s", h=2, s=6)[:, hf])
                for hf in range(2):
                    nc.vector.tensor_scalar_mul(pack[:, hf * 2:hf * 2 + 1],
                                                agg1[hf][:, 0:1], float(SH))
                    nc.scalar.square(sq[:], agg1[hf][:, 0:1])
                    nc.vector.tensor_add(pack[:, hf * 2 + 1:hf * 2 + 2],
                                         agg1[hf][:, 1:2], sq[:])
                    nc.vector.tensor_scalar_mul(pack[:, hf * 2 + 1:hf * 2 + 2],
                                                pack[:, hf * 2 + 1:hf * 2 + 2],
                                                float(SH))
                res = allreduce_stats(pack)
                sc2, bi2 = [], []
                for hf in range(2):
                    mu = wk.tile([128, 1], f32, tag=f"mu2{hf}")
                    nc.vector.tensor_scalar_mul(mu[:], res[:, hf * 2:hf * 2 + 1], 1.0 / N)
                    var = wk.tile([128, 1], f32, tag=f"var2{hf}")
                    nc.vector.tensor_scalar_mul(var[:], res[:, hf * 2 + 1:hf * 2 + 2], 1.0 / N)
                    nc.scalar.square(sq[:], mu[:])
                    nc.vector.tensor_tensor(out=var[:], in0=var[:], in1=sq[:], op=OP.subtract)
                    sd = wk.tile([128, 1], f32, tag=f"sd2{hf}")
                    nc.scalar.activation(sd[:], var[:], AF.Sqrt, bias=bneps_t[:])
                    rinv = wk.tile([128, 1], f32, tag=f"ri2{hf}")
                    nc.vector.reciprocal(rinv[:], sd[:])
                    s_ = wk.tile([128, 1], f32, tag=f"s2{hf}")
                    nc.vector.tensor_mul(s_[:], rinv[:], col(l, 8 + hf))
                    mb = wk.tile([128, 1], f32, tag=f"mb2{hf}")
                    nc.vector.tensor_add(mb[:], mu[:], col(l, 6 + hf))
                    ns = wk.tile([128, 1], f32, tag=f"ns2{hf}")
                    nc.vector.tensor_scalar_mul(ns[:], s_[:], -1.0)
                    b_ = wk.tile([128, 1], f32, tag=f"b2{hf}")
                    nc.vector.scalar_tensor_tensor(
                        out=b_[:], in0=mb[:], scalar=ns[:], in1=col(l, 10 + hf),
                        op0=OP.mult, op1=OP.add)
                    sc2.append(s_)
                    bi2.append(b_)

                # pass C: full forward, h update
                for i, (s, w) in enumerate(MT):
                    a1t = [None, None]
                    for hf in range(2):
                        ps = psB.tile([128, 512], f32, tag="z1")
                        nc.tensor.matmul(ps[:, :w], lhsT=W1l[:, hf * 128:(hf + 1) * 128],
                                         rhs=hh16[:, s:s + w], start=True, stop=True)
                        at = wk.tile([128, 512], f16, tag=f"a1_{hf}", bufs=2)
                        nc.scalar.activation(at[:, :w], ps[:, :w], AF.Relu,
                                             bias=bi1[hf][:], scale=sc1[hf][:])
                        a1t[hf] = at
                    a2t = [None, None]
                    for ho in range(2):
                        ps = psB.tile([128, 512], f32, tag="z2")
                        for kh in range(2):
                            nc.tensor.matmul(ps[:, :w],
                                             lhsT=W2l[kh][:, ho * 128:(ho + 1) * 128],
                                             rhs=a1t[kh][:, :w],
                                             start=(kh == 0), stop=(kh == 1))
                        at = wk.tile([128, 512], f16, tag=f"a2_{ho}", bufs=2)
                        nc.scalar.activation(at[:, :w], ps[:, :w], AF.Relu,
                                             bias=bi2[ho][:], scale=sc2[ho][:])
                        a2t[ho] = at
                    ps = psB.tile([128, 512], f32, tag="z3")
                    for kh in range(2):
                        nc.tensor.matmul(ps[:, :w], lhsT=W3l[kh][:],
                                         rhs=a2t[kh][:, :w],
                                         start=(kh == 0), stop=(kh == 1))
                    if l == 0:
                        nc.vector.tensor_scalar(
                            out=h_c[:, s:s + w], in0=ps[:, :w],
                            scalar1=col(l, 12), scalar2=None, op0=OP.add)
                    else:
                        nc.vector.scalar_tensor_tensor(
                            out=h_c[:, s:s + w], in0=ps[:, :w], scalar=col(l, 12),
                            in1=h_c[:, s:s + w], op0=OP.add, op1=OP.add)
                    if l < nl - 1 and s < SH:
                        wreal = min(s + w, SH) - s
                        nc.vector.bn_stats(stb_n[:, i * 6:(i + 1) * 6],
                                           h_c[:, s:s + wreal])

            nc.gpsimd.dma_start(out=hT_out[:], in_=h_c[:])

    nc.compile()
    return nc


# ---------------------------------------------------------------- runner

def _get_runner(nc):
    """Cached jitted SPMD executor for nc (axon PJRT path)."""
    import jax
    from jax.sharding import Mesh, PartitionSpec
    from jax.experimental.shard_map import shard_map
    import concourse.mybir as mybir
    from concourse.bass2jax import _bass_exec_p, install_neuronx_cc_hook, \
        partition_id_tensor

    install_neuronx_cc_hook()
    partition_name = nc.partition_id_tensor.name if nc.partition_id_tensor else None
    in_names, out_names, out_avals, zero_outs = [], [], [], []
    for alloc in nc.m.functions[0].allocations:
        if not isinstance(alloc, mybir.MemoryLocationSet):
            continue
        name = alloc.memorylocations[0].name
        if alloc.kind == "ExternalInput":
            if name != partition_name:
                in_names.append(name)
        elif alloc.kind == "ExternalOutput":
            out_names.append(name)
            shape = tuple(alloc.tensor_shape)
            dtype = mybir.dt.np(alloc.dtype)
            out_avals.append(jax.core.ShapedArray(shape, dtype))
            zero_outs.append(np.zeros(shape, dtype))
    n_params = len(in_names)
    n_outs = len(out_avals)
    in_names = in_names + out_names
    if partition_name is not None:
        in_names.append(partition_name)

    def _body(*args):
        operands = list(args)
        if partition_name is not None:
            operands.append(partition_id_tensor())
        outs = _bass_exec_p.bind(
            *operands, out_avals=tuple(out_avals), in_names=tuple(in_names),
            out_names=tuple(out_names), lowering_input_output_aliases=(),
            sim_require_finite=True, sim_require_nnan=True, nc=nc)
        return tuple(outs)

    devices = jax.devices()[:NCORES]
    mesh = Mesh(np.asarray(devices), ("core",))
    donate = tuple(range(n_params, n_params + n_outs))
    sharded = jax.jit(
        shard_map(_body, mesh=mesh,
                  in_specs=(PartitionSpec("core"),) * (n_params + n_outs),
                  out_specs=(PartitionSpec("core"),) * len(out_names),
                  check_rep=False),
        donate_argnums=donate, keep_unused=True)
    return dict(fn=sharded, in_names=in_names[:n_params], out_names=out_names,
                out_avals=out_avals, zero_outs=zero_outs)


def _run(runner, in_maps, timing_reps=1):
    import jax, time
    global LAST_HW_EXEC_NS
    concat_in = [np.concatenate([np.asarray(in_maps[c][nm]) for c in range(NCORES)],
                                axis=0)
                 for nm in runner["in_names"]]

    def one_call():
        zeros = [np.zeros((NCORES * z.shape[0], *z.shape[1:]), z.dtype)
                 for z in runner["zero_outs"]]
        out = runner["fn"](*concat_in, *zeros)
        jax.block_until_ready(out)
        return out

    out_arrs = one_call()
    times = []
    for _ in range(timing_reps):
        t0 = time.time()
        one_call()
        times.append(time.time() - t0)
    if times:
        LAST_HW_EXEC_NS = int(min(times) * 1e9)
    results = []
    for c in range(NCORES):
        res = {}
        for i, nm in enumerate(runner["out_names"]):
            shp = runner["out_avals"][i].shape
            res[nm] = np.asarray(out_arrs[i]).reshape(NCORES, *shp)[c]
        results.append(res)
    return results


# ---------------------------------------------------------------- entry

def kernel(x, edge_index, edge_attr, batch, atom_emb, bond_emb, W1, b1, g1, be1,
           W2, b2, g2, be2, W3, b3, t, norm_g, norm_b, predW, predb):
    x = np.asarray(x); edge_index = np.asarray(edge_index)
    edge_attr = np.asarray(edge_attr); batch = np.asarray(batch)
    atom_emb = np.asarray(atom_emb, np.float32)
    bond_emb = np.asarray(bond_emb, np.float32)
    W1 = np.asarray(W1, np.float32); b1 = np.asarray(b1, np.float32)
    g1 = np.asarray(g1, np.float32); be1 = np.asarray(be1, np.float32)
    W2 = np.asarray(W2, np.float32); b2 = np.asarray(b2, np.float32)
    g2 = np.asarray(g2, np.float32); be2 = np.asarray(be2, np.float32)
    W3 = np.asarray(W3, np.float32); b3 = np.asarray(b3, np.float32)
    t = np.asarray(t, np.float32)
    norm_g = np.asarray(norm_g, np.float32); norm_b = np.asarray(norm_b, np.float32)
    predW = np.asarray(predW, np.float32); predb = np.asarray(predb, np.float32)
    assert np.all(t > 0), "kernel assumes positive softmax temperature"

    nl = NL
    pre = _preprocess(x, edge_index, edge_attr, atom_emb)
    wblob, emb8, pblob = _pack_params(
        bond_emb, W1, W2, W3, b1, g1, be1, b2, g2, be2, b3, norm_g, norm_b, nl)

    key = (nl, pre["nchunk"], tuple(pre["ksched"]), tuple(np.round(t, 7)))
    if key not in _CACHE:
        nc = _build(nl, pre["nchunk"], pre["ksched"], t)
        _CACHE[key] = (nc, _get_runner(nc))
    nc, runner = _CACHE[key]

    in_maps = []
    for c in range(NCORES):
        in_maps.append(dict(
            h0T=pre["h0T"][c].astype(np.float16), offs=pre["offs2"][c], ecnt=pre["ecnt"][c],
            wblob=wblob, emb8=emb8, pblob=pblob))
    res = _run(runner, in_maps)

    # host: un-permute, final BN, pooling
    h = np.empty((N, D), np.float32)
    for c in range(NCORES):
        hT = res[c]["hT_out"].astype(np.float32)
        hloc = np.empty((SH, D), np.float32)
        hloc[pre["perms"][c]] = hT.T[:SH]
        h[c * SH:(c + 1) * SH] = hloc
    mu = h.mean(0)
    var = h.var(0)
    h = (h - mu) / np.sqrt(var + BN_EPS) * norm_g[nl - 1] + norm_b[nl - 1]
    cnt = np.zeros(G, np.float32)
    np.add.at(cnt, batch, 1.0)
    sums = np.zeros((G, D), np.float32)
    np.add.at(sums, batch, h)
    hg = sums / np.maximum(cnt, 1.0)[:, None]
    return (hg @ predW + predb).astype(np.float32)
